# revision 23
# baseline (speedup 1.0000x reference)
"""DRGNet (GNN message passing) Trainium2 kernel.

Strategy (8 NeuronCores, SPMD single program):
- Nodes partitioned across cores (12500/core, padded to 12544 = 196 windows x 64).
- Edges partitioned by destination core; per core grouped by (dst window 64,
  src range 25088, block 128) with fixed static shape: 3 blocks per
  (window, range) -> 12 blocks = 1536 edge slots per window.
- Per layer: dma_gather (int16 MoE gather) pulls h[src] rows (256B) from the
  full replicated node table in HBM; DVE builds a one-hot [128e, 64d] from
  rel-dst values vs a static iota; PE accumulates agg^T = msgs^T @ OH in PSUM
  (f32); transform (W_rel/W_root matmuls, feat-major), ELU composed as
  relu(z) + exp(min(z,0)) - 1; h tile transposed and written to staging.
- Per layer, two AllGathers (half-shard each) rebuild the replicated table for
  the next layer's gathers; node numbering is chosen so the AllGather output
  is exactly the table layout (row = half*50176 + core*6272 + j).
- Dense sort-pool/conv head runs on host (tiny; selection is data-dependent).

Everything is f32: the sort-pool top-k selection tolerates only ~1e-6 feature
error (measured), which rules out bf16 anywhere on the main path.
"""
import os
import numpy as np

# ---------------- constants ----------------
N_NODES = 100000
N_EDGES = 1600000
D = 64                      # feature dim
NC = 8                      # cores
NPC_REAL = 12500            # real nodes per core
W = 64                      # dst window size
WPC = 196                   # windows per core (12544 nodes padded)
NPC = WPC * W               # 12544
NTOT = NC * NPC             # 100352 table rows
HALF = NPC // 2             # 6272 (AllGather half-shard)
NRANGE = 4                  # int16 src ranges
RANGE = NTOT // NRANGE      # 25088 rows per range
BPR = 3                     # blocks per (window, range)
BPW = NRANGE * BPR          # 12 blocks per window
SLOTW = BPW * 128           # 1536 edge slots per window
GRP = 7                     # windows per gather group
NGRP = WPC // GRP           # 28 groups
CALL_IDX = GRP * BPR * 128  # 2688 idxs per gather call
NSLOT = WPC * SLOTW         # 301056 slots per core per layer
NLAYERS = 5
K = 30
NUM_CLASSES = 5

_cache = {}


# ---------------- host prep ----------------
def _prep_edges(edge_index, edge_weight):
    """Slot every edge into (core, window, range, block, partition).

    Returns per-core arrays:
      idx16   [128, NSLOT//16] int16  (wrapped 16-partition, replicated x8)
      reldst  [128, WPC*BPW] f32      (dst slot 0..63 per (w, r*3+b) col)
      ew      [128, WPC*BPW... same layout as msgs blocks]
    """
    src = np.asarray(edge_index[0], dtype=np.int64)
    dst = np.asarray(edge_index[1], dtype=np.int64)
    ew = np.asarray(edge_weight, dtype=np.float32)

    c = dst // NPC_REAL
    local_d = dst - c * NPC_REAL
    w = local_d // W
    s = (local_d % W).astype(np.float32)

    cs = src // NPC_REAL
    ls = src - cs * NPC_REAL
    hs = (ls >= HALF).astype(np.int64)
    srow = hs * (NC * HALF) + cs * HALF + (ls - hs * HALF)
    r = srow // RANGE
    rel = (srow - r * RANGE).astype(np.int16)

    # group id per edge and position within group
    gid = ((c * WPC + w) * NRANGE + r).astype(np.int64)
    order = np.argsort(gid, kind="stable")
    gid_s = gid[order]
    # position within each group
    pos = np.arange(N_EDGES, dtype=np.int64)
    grp_start = np.zeros(N_EDGES, dtype=np.int64)
    first = np.ones(N_EDGES, dtype=bool)
    first[1:] = gid_s[1:] != gid_s[:-1]
    grp_start[first] = pos[first]
    grp_start = np.maximum.accumulate(grp_start)
    pos_in = pos - grp_start
    cap = BPR * 128
    counts = np.bincount(gid, minlength=NC * WPC * NRANGE)
    if counts.max() > cap:
        raise RuntimeError(f"(window, range) overflow: {counts.max()} > {cap}; "
                           "need node rebalancing")

    # flat slot within core: layout [g 28][r 4][w_in_g 7][b 3][p 128]
    w_s = w[order]
    r_s = r[order]
    g_s = w_s // GRP
    wig = w_s % GRP
    b = pos_in // 128
    p = pos_in % 128
    flat = ((((g_s * NRANGE + r_s) * GRP + wig) * BPR + b) * 128 + p)
    core_s = c[order]

    idx16 = np.zeros((NC, NSLOT), dtype=np.int16)
    relw = np.zeros((NC, WPC * BPW, 128), dtype=np.float32)
    eww = np.zeros((NC, WPC * BPW, 128), dtype=np.float32)
    col = (w_s * BPW + r_s * BPR + b)
    for cc in range(NC):
        m = core_s == cc
        idx16[cc, flat[m]] = rel[order][m]
        relw[cc, col[m], p[m]] = s[order][m]
        eww[cc, col[m], p[m]] = ew[order][m]

    # wrap idx per call: call k occupies idx slots [k*2688, (k+1)*2688)
    ncall = NSLOT // CALL_IDX
    idx_wrapped = np.zeros((NC, 128, NSLOT // 16), dtype=np.int16)
    for cc in range(NC):
        v = idx16[cc].reshape(ncall, CALL_IDX // 16, 16)
        wv = v.transpose(0, 2, 1).reshape(ncall, 16, CALL_IDX // 16)
        flat_w = np.concatenate([wv[k] for k in range(ncall)], axis=1)
        idx_wrapped[cc] = np.tile(flat_w, (8, 1))
    # reldst/ew as [128, cols]
    relw = relw.transpose(0, 2, 1).copy()
    eww = eww.transpose(0, 2, 1).copy()
    return idx_wrapped, relw, eww


def _node_tables(x):
    """x [100000, 64] -> per-core (x_stage [NPC, 64], xT_own [64, NPC])."""
    xs, xT = [], []
    for cc in range(NC):
        xl = np.zeros((NPC, D), dtype=np.float32)
        xl[:NPC_REAL] = x[cc * NPC_REAL:(cc + 1) * NPC_REAL]
        xs.append(xl)
        xT.append(np.ascontiguousarray(xl.T))
    return xs, xT


# ---------------- bass program ----------------
def _build_program(collectives=True, nlayers=NLAYERS):
    import concourse.bacc as bacc
    import concourse.mybir as mybir
    import concourse.tile as tile
    from concourse.bass import AP

    f32 = mybir.dt.float32
    i16 = mybir.dt.int16
    Alu = mybir.AluOpType
    Act = mybir.ActivationFunctionType

    nc = bacc.Bacc("TRN2", target_bir_lowering=False, debug=False,
                   num_devices=NC)

    x_stage = nc.dram_tensor("x_stage", [NPC, D], f32, kind="ExternalInput")
    xT_own = nc.dram_tensor("xT_own", [D, NPC], f32, kind="ExternalInput")
    idx_in = nc.dram_tensor("idx16", [128, NSLOT // 16], i16, kind="ExternalInput")
    rel_in = nc.dram_tensor("reldst", [128, WPC * BPW], f32, kind="ExternalInput")
    ew_in = nc.dram_tensor("eww", [128, WPC * BPW], f32, kind="ExternalInput")
    iota_in = nc.dram_tensor("iota", [128, W], f32, kind="ExternalInput")
    ident_in = nc.dram_tensor("ident64", [D, D], f32, kind="ExternalInput")
    wrel_in, wroot_in, b_in = [], [], []
    for L in range(nlayers):
        od = D if L < 4 else 1
        wrel_in.append(nc.dram_tensor(f"wrelT{L}", [D, od], f32, kind="ExternalInput"))
        wroot_in.append(nc.dram_tensor(f"wrootT{L}", [D, od], f32, kind="ExternalInput"))
        b_in.append(nc.dram_tensor(f"bias{L}", [od, 1], f32, kind="ExternalInput"))
    h_out = [nc.dram_tensor(f"h{L}_out", [NPC, D] if L < 4 else [1, NPC], f32,
                            kind="ExternalOutput") for L in range(nlayers)]

    with tile.TileContext(nc) as tc:
        with tc.tile_pool(name="const", bufs=1) as cpool, \
             tc.tile_pool(name="msgs", bufs=2) as mpool, \
             tc.tile_pool(name="oh", bufs=4) as ohpool, \
             tc.tile_pool(name="sc", bufs=4) as scpool, \
             tc.tile_pool(name="work", bufs=3) as wpool, \
             tc.tile_pool(name="psum", bufs=2, space="PSUM") as ppool, \
             tc.tile_pool(name="dram", bufs=1, space="DRAM") as dpool:

            idx_t = cpool.tile([128, NSLOT // 16], i16, tag="idx")
            nc.sync.dma_start(out=idx_t[:], in_=idx_in[:, :])
            rel_t = cpool.tile([128, WPC * BPW], f32, tag="rel")
            nc.sync.dma_start(out=rel_t[:], in_=rel_in[:, :])
            ew_t = cpool.tile([128, WPC * BPW], f32, tag="ew")
            nc.sync.dma_start(out=ew_t[:], in_=ew_in[:, :])
            iota_t = cpool.tile([128, W], f32, tag="iota")
            nc.sync.dma_start(out=iota_t[:], in_=iota_in[:, :])
            ident_t = cpool.tile([D, D], f32, tag="ident")
            nc.sync.dma_start(out=ident_t[:], in_=ident_in[:, :])
            wrel_t, wroot_t, bias_t = [], [], []
            for L in range(nlayers):
                od = D if L < 4 else 1
                wt = cpool.tile([D, od], f32, tag=f"wrel{L}")
                nc.sync.dma_start(out=wt[:], in_=wrel_in[L][:, :])
                wrel_t.append(wt)
                wt = cpool.tile([D, od], f32, tag=f"wroot{L}")
                nc.sync.dma_start(out=wt[:], in_=wroot_in[L][:, :])
                wroot_t.append(wt)
                bt = cpool.tile([od, 1], f32, tag=f"bias{L}")
                nc.sync.dma_start(out=bt[:], in_=b_in[L][:, :])
                bias_t.append(bt)

            # each table = two half tiles [50176, D]; range r is inside half r//2
            tables = [(dpool.tile([NC * HALF, D], f32, tag=f"tabA{L}",
                                  name=f"tabA{L}"),
                       dpool.tile([NC * HALF, D], f32, tag=f"tabB{L}",
                                  name=f"tabB{L}"))
                      for L in range(nlayers)]          # tables for x, h1..h4
            stages = [dpool.tile([NPC, D], f32, tag=f"stage{L}",
                                 name=f"stage{L}")
                      for L in range(nlayers - 1)]      # node-major h1..h4
            h5_stage = dpool.tile([1, NPC], f32, tag="h5stage")
            hT_own = [dpool.tile([D, NPC], f32, tag=f"hT{L}", name=f"hT{L}")
                      for L in range(nlayers - 1)]      # feat-major own h1..h4

            def src_range_ap(L, r):
                """[RANGE, D] AP for src range r of layer-L input table."""
                half_t = tables[L][r // 2][:]
                return AP(half_t.tensor,
                          half_t.offset + (r % 2) * RANGE * D,
                          [[D, RANGE], [1, D]])

            # build the layer-0 table from sharded x via two AllGathers
            # (bounce through an internal DRAM tile: collectives cannot
            # source I/O tensors)
            x_bounce = dpool.tile([NPC, D], f32, tag="xb", name="x_bounce")
            nc.gpsimd.dma_start(out=x_bounce[:], in_=x_stage.ap())
            _ag(nc, collectives, x_bounce, tables[0], 0, dpool, -1)
            _ag(nc, collectives, x_bounce, tables[0], 1, dpool, -1)

            def xT_ap(L, t):
                base = xT_own if L == 0 else hT_own[L - 1]
                if L == 0:
                    return base[:, t * 128:(t + 1) * 128]
                return base[:, t * 128:(t + 1) * 128]

            for L in range(nlayers):
                od = D if L < 4 else 1
                aggT = None
                for g in range(NGRP):
                    msgs = mpool.tile([128, NRANGE * GRP * BPR * D], f32, tag="m")
                    for r in range(NRANGE):
                        tab_r = src_range_ap(L, r)
                        call = g * NRANGE + r
                        o = msgs[:, r * GRP * BPR * D:(r + 1) * GRP * BPR * D]
                        nc.gpsimd.dma_gather(
                            o.rearrange("p (k d) -> p k d", d=D),
                            tab_r,
                            idx_t[:, call * (CALL_IDX // 16):(call + 1) * (CALL_IDX // 16)],
                            CALL_IDX, CALL_IDX, D,
                            single_packet=False)
                    for wi in range(GRP):
                        wg = g * GRP + wi
                        oh = ohpool.tile([128, BPW * W], f32, tag="oh")
                        rel_ap = rel_t[:, wg * BPW:(wg + 1) * BPW]
                        in_rel = AP(rel_ap.tensor, rel_ap.offset,
                                    [rel_ap.ap[0], [1, BPW], [0, W]])
                        in_iota = AP(iota_t[:].tensor, iota_t[:].offset,
                                     [iota_t[:].ap[0], [0, BPW], [1, W]])
                        oh_ap = oh[:]
                        out_oh = AP(oh_ap.tensor, oh_ap.offset,
                                    [oh_ap.ap[0], [W, BPW], [1, W]])
                        nc.vector.tensor_tensor(out=out_oh, in0=in_iota,
                                                in1=in_rel, op=Alu.is_equal)
                        sc = scpool.tile([128, BPW * D], f32, tag="sc")
                        m_ap = msgs[:]
                        in_m = AP(m_ap.tensor, m_ap.offset + wi * BPR * D,
                                  [m_ap.ap[0], [GRP * BPR * D, NRANGE],
                                   [D, BPR], [1, D]])
                        ew_ap = ew_t[:, wg * BPW:(wg + 1) * BPW]
                        in_ew = AP(ew_ap.tensor, ew_ap.offset,
                                   [ew_ap.ap[0], [BPR, NRANGE], [1, BPR], [0, D]])
                        sc_ap = sc[:]
                        out_sc = AP(sc_ap.tensor, sc_ap.offset,
                                    [sc_ap.ap[0], [BPR * D, NRANGE],
                                     [D, BPR], [1, D]])
                        nc.vector.tensor_tensor(out=out_sc, in0=in_m, in1=in_ew,
                                                op=Alu.mult)
                        psum_w = ppool.tile([D, W], f32, tag="agg")
                        for j in range(BPW):
                            nc.tensor.matmul(
                                out=psum_w[:],
                                lhsT=sc[:, j * D:(j + 1) * D],
                                rhs=oh[:, j * W:(j + 1) * W],
                                start=(j == 0), stop=(j == BPW - 1))
                        if wg % 2 == 0:
                            aggT = wpool.tile([D, 128], f32, tag="aggT")
                        nc.vector.tensor_copy(
                            out=aggT[:, (wg % 2) * W:(wg % 2 + 1) * W],
                            in_=psum_w[:])
                        if wg % 2 == 0:
                            continue
                        # ---- transform + elu for tile t ----
                        t = wg // 2
                        xT = wpool.tile([D, 128], f32, tag="xT")
                        nc.sync.dma_start(out=xT[:], in_=xT_ap(L, t))
                        pt = ppool.tile([od, 128], f32, tag="tr")
                        nc.tensor.matmul(out=pt[:], lhsT=wrel_t[L][:],
                                         rhs=aggT[:], start=True, stop=False)
                        nc.tensor.matmul(out=pt[:], lhsT=wroot_t[L][:],
                                         rhs=xT[:], start=False, stop=True)
                        mm = wpool.tile([od, 128], f32, tag="mm")
                        nc.vector.tensor_scalar(out=mm[:], in0=pt[:],
                                             scalar1=bias_t[L][:], scalar2=0.0,
                                             op0=Alu.add, op1=Alu.min)
                        ee = wpool.tile([od, 128], f32, tag="ee")
                        nc.scalar.activation(out=ee[:], in_=mm[:], func=Act.Exp)
                        rr = wpool.tile([od, 128], f32, tag="rr")
                        nc.vector.tensor_scalar(out=rr[:], in0=pt[:],
                                             scalar1=bias_t[L][:], scalar2=0.0,
                                             op0=Alu.add, op1=Alu.max)
                        hT = wpool.tile([od, 128], f32, tag="hT")
                        nc.vector.tensor_tensor(out=hT[:], in0=ee[:], in1=rr[:],
                                                op=Alu.add)
                        nc.vector.tensor_scalar(out=hT[:], in0=hT[:], scalar1=-1.0,
                                             scalar2=None, op0=Alu.add)
                        if L < 4:
                            nc.sync.dma_start(
                                out=hT_own[L][:, t * 128:(t + 1) * 128],
                                in_=hT[:])
                            tp = ppool.tile([128, D], f32, tag="tp")
                            nc.tensor.transpose(out=tp[:], in_=hT[:],
                                                identity=ident_t[:])
                            hn = wpool.tile([128, D], f32, tag="hn")
                            nc.vector.tensor_copy(out=hn[:], in_=tp[:])
                            nc.sync.dma_start(
                                out=stages[L][t * 128:(t + 1) * 128, :],
                                in_=hn[:])
                        else:
                            nc.sync.dma_start(
                                out=h5_stage[:, t * 128:(t + 1) * 128],
                                in_=hT[:])
                        # fire AllGather halves as soon as available
                        if L < 4 and t == WPC // 4 - 1:
                            _ag(nc, collectives, stages[L][:, :], tables[L + 1],
                                0, dpool, L)
                        if L < 4 and t == WPC // 2 - 1:
                            _ag(nc, collectives, stages[L][:, :], tables[L + 1],
                                1, dpool, L)
                # end groups
            for L in range(4):
                nc.gpsimd.dma_start(out=h_out[L][:, :], in_=stages[L][:])
            nc.gpsimd.dma_start(out=h_out[4][:, :], in_=h5_stage[:])

    nc.compile()
    return nc


def _ag(nc, collectives, stage, table_halves, half, dpool, L):
    """AllGather stage rows [half*6272,(half+1)*6272) -> full half-table."""
    import concourse.mybir as mybir
    rows = HALF
    src = stage[half * rows:(half + 1) * rows, :]
    dst_tile = table_halves[half]
    if collectives:
        nc.gpsimd.collective_compute(
            "AllGather", mybir.AluOpType.bypass,
            replica_groups=[list(range(NC))],
            ins=[src.opt()], outs=[dst_tile.opt()])
    else:
        # sim mode: local copy of own shard (timing proxy, wrong data)
        nc.gpsimd.dma_start(out=dst_tile[:rows, :], in_=src)


# ---------------- head (numpy) ----------------
def _elu(x):
    return np.where(x > 0, x, np.expm1(x))


def _head(xcat, batch, num_graphs, conv1_w, conv1_b, conv2_w, conv2_b,
          mlp_w1, mlp_b1, mlp_w2, mlp_b2):
    n, d = xcat.shape
    perm = np.lexsort((-xcat[:, -1], batch))
    xs = xcat[perm]
    bs = batch[perm]
    counts = np.zeros(num_graphs, np.int64)
    np.add.at(counts, batch, 1)
    starts = np.cumsum(counts) - counts
    rank = np.arange(n) - starts[bs]
    slot = np.minimum(rank, K)
    out = np.zeros((num_graphs, K + 1, d), xcat.dtype)
    out[bs, slot] = xs
    xp = out[:, :K]
    h = _elu(np.einsum('bkd,od->bok', xp, conv1_w) + conv1_b[None, :, None])
    b_, c_, l_ = h.shape
    h = h.reshape(b_, c_, l_ // 2, 2).max(axis=-1)
    B, C, L = h.shape
    O, I, KK = conv2_w.shape
    outc = np.zeros((B, O, L - KK + 1), np.float32)
    for k in range(KK):
        outc += np.einsum('bcl,oc->bol', h[:, :, k:k + L - KK + 1],
                          conv2_w[:, :, k])
    h = _elu(outc + conv2_b[None, :, None])
    h = h.reshape(num_graphs, -1)
    h = _elu(h @ mlp_w1.T + mlp_b1)
    return (h @ mlp_w2.T + mlp_b2).astype(np.float32)


# ---------------- entry point ----------------
def kernel(x, edge_index, batch, edge_weight, num_graphs, gc_w_rel, gc_b_rel,
           gc_w_root, conv1_w, conv1_b, conv2_w, conv2_b, mlp_w1, mlp_b1,
           mlp_w2, mlp_b2):
    from concourse import bass_utils

    x = np.asarray(x, dtype=np.float32)
    batch_np = np.asarray(batch, dtype=np.int64)
    ng = int(num_graphs)

    idx_w, relw, eww = _prep_edges(np.asarray(edge_index), np.asarray(edge_weight))
    xs, xT = _node_tables(x)
    iota = np.tile(np.arange(W, dtype=np.float32)[None, :], (128, 1))

    if "nc" not in _cache:
        _cache["nc"] = _build_program(collectives=True)
    nc = _cache["nc"]

    in_maps = []
    for cc in range(NC):
        m = dict(x_stage=xs[cc], xT_own=xT[cc], idx16=idx_w[cc],
                 reldst=relw[cc], eww=eww[cc], iota=iota,
                 ident64=np.eye(D, dtype=np.float32))
        for L in range(NLAYERS):
            m[f"wrelT{L}"] = np.ascontiguousarray(
                np.asarray(gc_w_rel[L], np.float32).T)
            m[f"wrootT{L}"] = np.ascontiguousarray(
                np.asarray(gc_w_root[L], np.float32).T)
            m[f"bias{L}"] = np.asarray(gc_b_rel[L], np.float32)[:, None]
        in_maps.append(m)

    res = bass_utils.run_bass_kernel_spmd(nc, in_maps,
                                          core_ids=list(range(NC)))

    # assemble xcat [100000, 257]
    xcat = np.zeros((N_NODES, 4 * D + 1), dtype=np.float32)
    for cc in range(NC):
        r = res.results[cc]
        sl = slice(cc * NPC_REAL, (cc + 1) * NPC_REAL)
        for L in range(4):
            xcat[sl, L * D:(L + 1) * D] = r[f"h{L}_out"][:NPC_REAL]
        xcat[sl, 4 * D] = r["h4_out"][0, :NPC_REAL]

    out = _head(xcat, batch_np, ng,
                np.asarray(conv1_w, np.float32), np.asarray(conv1_b, np.float32),
                np.asarray(conv2_w, np.float32), np.asarray(conv2_b, np.float32),
                np.asarray(mlp_w1, np.float32), np.asarray(mlp_b1, np.float32),
                np.asarray(mlp_w2, np.float32), np.asarray(mlp_b2, np.float32))
    return out


# revision 26
# speedup vs baseline: 1.0161x; 1.0161x over previous
"""DRGNet (GNN message passing) Trainium2 kernel.

Strategy (8 NeuronCores, SPMD single program):
- Nodes partitioned across cores (12500/core, padded to 12544 = 196 windows x 64).
- Edges partitioned by destination core; per core grouped by (dst window 64,
  src range 25088, block 128) with fixed static shape: 3 blocks per
  (window, range) -> 12 blocks = 1536 edge slots per window.
- Per layer: dma_gather (int16 MoE gather) pulls h[src] rows (256B) from the
  full replicated node table in HBM; DVE builds a one-hot [128e, 64d] from
  rel-dst values vs a static iota; PE accumulates agg^T = msgs^T @ OH in PSUM
  (f32); transform (W_rel/W_root matmuls, feat-major), ELU composed as
  relu(z) + exp(min(z,0)) - 1; h tile transposed and written to staging.
- Per layer, two AllGathers (half-shard each) rebuild the replicated table for
  the next layer's gathers; node numbering is chosen so the AllGather output
  is exactly the table layout (row = half*50176 + core*6272 + j).
- Dense sort-pool/conv head runs on host (tiny; selection is data-dependent).

Everything is f32: the sort-pool top-k selection tolerates only ~1e-6 feature
error (measured), which rules out bf16 anywhere on the main path.
"""
import os
import numpy as np

# ---------------- constants ----------------
N_NODES = 100000
N_EDGES = 1600000
D = 64                      # feature dim
NC = 8                      # cores
NPC_REAL = 12500            # real nodes per core
W = 64                      # dst window size
WPC = 196                   # windows per core (12544 nodes padded)
NPC = WPC * W               # 12544
NTOT = NC * NPC             # 100352 table rows
HALF = NPC // 2             # 6272 (AllGather half-shard)
NRANGE = 4                  # int16 src ranges
RANGE = NTOT // NRANGE      # 25088 rows per range
BPR = 3                     # blocks per (window, range)
BPW = NRANGE * BPR          # 12 blocks per window
SLOTW = BPW * 128           # 1536 edge slots per window
GRP = 7                     # windows per gather group
NGRP = WPC // GRP           # 28 groups
CALL_IDX = GRP * BPR * 128  # 2688 idxs per gather call
NSLOT = WPC * SLOTW         # 301056 slots per core per layer
NLAYERS = 5
K = 30
NUM_CLASSES = 5

_cache = {}


# ---------------- host prep ----------------
def _prep_edges(edge_index, edge_weight):
    """Slot every edge into (core, window, range, block, partition).

    Returns per-core arrays:
      idx16   [128, NSLOT//16] int16  (wrapped 16-partition, replicated x8)
      reldst  [128, WPC*BPW] f32      (dst slot 0..63 per (w, r*3+b) col)
      ew      [128, WPC*BPW... same layout as msgs blocks]
    """
    src = np.asarray(edge_index[0], dtype=np.int64)
    dst = np.asarray(edge_index[1], dtype=np.int64)
    ew = np.asarray(edge_weight, dtype=np.float32)

    c = dst // NPC_REAL
    local_d = dst - c * NPC_REAL
    w = local_d // W
    s = (local_d % W).astype(np.float32)

    cs = src // NPC_REAL
    ls = src - cs * NPC_REAL
    hs = (ls >= HALF).astype(np.int64)
    srow = hs * (NC * HALF) + cs * HALF + (ls - hs * HALF)
    r = srow // RANGE
    rel = (srow - r * RANGE).astype(np.int16)

    # group id per edge and position within group
    gid = ((c * WPC + w) * NRANGE + r).astype(np.int64)
    order = np.argsort(gid, kind="stable")
    gid_s = gid[order]
    # position within each group
    pos = np.arange(N_EDGES, dtype=np.int64)
    grp_start = np.zeros(N_EDGES, dtype=np.int64)
    first = np.ones(N_EDGES, dtype=bool)
    first[1:] = gid_s[1:] != gid_s[:-1]
    grp_start[first] = pos[first]
    grp_start = np.maximum.accumulate(grp_start)
    pos_in = pos - grp_start
    cap = BPR * 128
    counts = np.bincount(gid, minlength=NC * WPC * NRANGE)
    if counts.max() > cap:
        raise RuntimeError(f"(window, range) overflow: {counts.max()} > {cap}; "
                           "need node rebalancing")

    # flat slot within core: layout [g 28][r 4][w_in_g 7][b 3][p 128]
    w_s = w[order]
    r_s = r[order]
    g_s = w_s // GRP
    wig = w_s % GRP
    b = pos_in // 128
    p = pos_in % 128
    flat = ((((g_s * NRANGE + r_s) * GRP + wig) * BPR + b) * 128 + p)
    core_s = c[order]

    idx16 = np.zeros((NC, NSLOT), dtype=np.int16)
    relw = np.zeros((NC, WPC * BPW, 128), dtype=np.float32)
    eww = np.zeros((NC, WPC * BPW, 128), dtype=np.float32)
    col = (w_s * BPW + r_s * BPR + b)
    for cc in range(NC):
        m = core_s == cc
        idx16[cc, flat[m]] = rel[order][m]
        relw[cc, col[m], p[m]] = s[order][m]
        eww[cc, col[m], p[m]] = ew[order][m]

    # wrap idx per call: call k occupies idx slots [k*2688, (k+1)*2688)
    ncall = NSLOT // CALL_IDX
    idx_wrapped = np.zeros((NC, 128, NSLOT // 16), dtype=np.int16)
    for cc in range(NC):
        v = idx16[cc].reshape(ncall, CALL_IDX // 16, 16)
        wv = v.transpose(0, 2, 1).reshape(ncall, 16, CALL_IDX // 16)
        flat_w = np.concatenate([wv[k] for k in range(ncall)], axis=1)
        idx_wrapped[cc] = np.tile(flat_w, (8, 1))
    # reldst/ew as [128, cols]
    relw = relw.transpose(0, 2, 1).copy()
    eww = eww.transpose(0, 2, 1).copy()
    return idx_wrapped, relw, eww


def _node_tables(x):
    """x [100000, 64] -> per-core (x_stage [NPC, 64], xT_own [64, NPC])."""
    xs, xT = [], []
    for cc in range(NC):
        xl = np.zeros((NPC, D), dtype=np.float32)
        xl[:NPC_REAL] = x[cc * NPC_REAL:(cc + 1) * NPC_REAL]
        xs.append(xl)
        xT.append(np.ascontiguousarray(xl.T))
    return xs, xT


# ---------------- bass program ----------------
def _build_program(collectives=True, nlayers=NLAYERS):
    import concourse.bacc as bacc
    import concourse.mybir as mybir
    import concourse.tile as tile
    from concourse.bass import AP

    f32 = mybir.dt.float32
    i16 = mybir.dt.int16
    Alu = mybir.AluOpType
    Act = mybir.ActivationFunctionType

    nc = bacc.Bacc("TRN2", target_bir_lowering=False, debug=False,
                   num_devices=NC)

    x_stage = nc.dram_tensor("x_stage", [NPC, D], f32, kind="ExternalInput")
    xT_own = nc.dram_tensor("xT_own", [D, NPC], f32, kind="ExternalInput")
    idx_in = nc.dram_tensor("idx16", [128, NSLOT // 16], i16, kind="ExternalInput")
    rel_in = nc.dram_tensor("reldst", [128, WPC * BPW], f32, kind="ExternalInput")
    ew_in = nc.dram_tensor("eww", [128, WPC * BPW], f32, kind="ExternalInput")
    iota_in = nc.dram_tensor("iota", [128, W], f32, kind="ExternalInput")
    ident_in = nc.dram_tensor("ident64", [D, D], f32, kind="ExternalInput")
    wrel_in, wroot_in, b_in = [], [], []
    for L in range(nlayers):
        od = D if L < 4 else 1
        wrel_in.append(nc.dram_tensor(f"wrelT{L}", [D, od], f32, kind="ExternalInput"))
        wroot_in.append(nc.dram_tensor(f"wrootT{L}", [D, od], f32, kind="ExternalInput"))
        b_in.append(nc.dram_tensor(f"bias{L}", [od, 1], f32, kind="ExternalInput"))
    h_out = [nc.dram_tensor(f"h{L}_out", [NPC, D] if L < 4 else [1, NPC], f32,
                            kind="ExternalOutput") for L in range(nlayers)]

    with tile.TileContext(nc) as tc:
        with tc.tile_pool(name="const", bufs=1) as cpool, \
             tc.tile_pool(name="msgs", bufs=3) as mpool, \
             tc.tile_pool(name="oh", bufs=4) as ohpool, \
             tc.tile_pool(name="sc", bufs=4) as scpool, \
             tc.tile_pool(name="work", bufs=3) as wpool, \
             tc.tile_pool(name="psum", bufs=2, space="PSUM") as ppool, \
             tc.tile_pool(name="dram", bufs=1, space="DRAM") as dpool:

            idx_t = cpool.tile([128, NSLOT // 16], i16, tag="idx")
            nc.sync.dma_start(out=idx_t[:], in_=idx_in[:, :])
            rel_t = cpool.tile([128, WPC * BPW], f32, tag="rel")
            nc.sync.dma_start(out=rel_t[:], in_=rel_in[:, :])
            ew_t = cpool.tile([128, WPC * BPW], f32, tag="ew")
            nc.sync.dma_start(out=ew_t[:], in_=ew_in[:, :])
            iota_t = cpool.tile([128, W], f32, tag="iota")
            nc.sync.dma_start(out=iota_t[:], in_=iota_in[:, :])
            ident_t = cpool.tile([D, D], f32, tag="ident")
            nc.sync.dma_start(out=ident_t[:], in_=ident_in[:, :])
            wrel_t, wroot_t, bias_t = [], [], []
            for L in range(nlayers):
                od = D if L < 4 else 1
                wt = cpool.tile([D, od], f32, tag=f"wrel{L}")
                nc.sync.dma_start(out=wt[:], in_=wrel_in[L][:, :])
                wrel_t.append(wt)
                wt = cpool.tile([D, od], f32, tag=f"wroot{L}")
                nc.sync.dma_start(out=wt[:], in_=wroot_in[L][:, :])
                wroot_t.append(wt)
                bt = cpool.tile([od, 1], f32, tag=f"bias{L}")
                nc.sync.dma_start(out=bt[:], in_=b_in[L][:, :])
                bias_t.append(bt)

            # each table = two half tiles [50176, D]; range r is inside half r//2
            tables = [(dpool.tile([NC * HALF, D], f32, tag=f"tabA{L}",
                                  name=f"tabA{L}"),
                       dpool.tile([NC * HALF, D], f32, tag=f"tabB{L}",
                                  name=f"tabB{L}"))
                      for L in range(nlayers)]          # tables for x, h1..h4
            stages = [dpool.tile([NPC, D], f32, tag=f"stage{L}",
                                 name=f"stage{L}")
                      for L in range(nlayers - 1)]      # node-major h1..h4
            h5_stage = dpool.tile([1, NPC], f32, tag="h5stage")
            hT_own = [dpool.tile([D, NPC], f32, tag=f"hT{L}", name=f"hT{L}")
                      for L in range(nlayers - 1)]      # feat-major own h1..h4

            def src_range_ap(L, r):
                """[RANGE, D] AP for src range r of layer-L input table."""
                half_t = tables[L][r // 2][:]
                return AP(half_t.tensor,
                          half_t.offset + (r % 2) * RANGE * D,
                          [[D, RANGE], [1, D]])

            # build the layer-0 table from sharded x via two AllGathers
            # (bounce through an internal DRAM tile: collectives cannot
            # source I/O tensors)
            x_bounce = dpool.tile([NPC, D], f32, tag="xb", name="x_bounce")
            nc.gpsimd.dma_start(out=x_bounce[:], in_=x_stage.ap())
            _ag(nc, collectives, x_bounce, tables[0], 0, dpool, -1)
            _ag(nc, collectives, x_bounce, tables[0], 1, dpool, -1)

            def xT_ap(L, t):
                base = xT_own if L == 0 else hT_own[L - 1]
                if L == 0:
                    return base[:, t * 128:(t + 1) * 128]
                return base[:, t * 128:(t + 1) * 128]

            for L in range(nlayers):
                od = D if L < 4 else 1
                aggT = None
                for g in range(NGRP):
                    msgs = mpool.tile([128, NRANGE * GRP * BPR * D], f32, tag="m")
                    for r in range(NRANGE):
                        tab_r = src_range_ap(L, r)
                        call = g * NRANGE + r
                        o = msgs[:, r * GRP * BPR * D:(r + 1) * GRP * BPR * D]
                        nc.gpsimd.dma_gather(
                            o.rearrange("p (k d) -> p k d", d=D),
                            tab_r,
                            idx_t[:, call * (CALL_IDX // 16):(call + 1) * (CALL_IDX // 16)],
                            CALL_IDX, CALL_IDX, D,
                            single_packet=False)
                    for wi in range(GRP):
                        wg = g * GRP + wi
                        oh = ohpool.tile([128, BPW * W], f32, tag="oh")
                        rel_ap = rel_t[:, wg * BPW:(wg + 1) * BPW]
                        in_rel = AP(rel_ap.tensor, rel_ap.offset,
                                    [rel_ap.ap[0], [1, BPW], [0, W]])
                        in_iota = AP(iota_t[:].tensor, iota_t[:].offset,
                                     [iota_t[:].ap[0], [0, BPW], [1, W]])
                        oh_ap = oh[:]
                        out_oh = AP(oh_ap.tensor, oh_ap.offset,
                                    [oh_ap.ap[0], [W, BPW], [1, W]])
                        nc.vector.tensor_tensor(out=out_oh, in0=in_iota,
                                                in1=in_rel, op=Alu.is_equal)
                        sc = scpool.tile([128, BPW * D], f32, tag="sc")
                        m_ap = msgs[:]
                        in_m = AP(m_ap.tensor, m_ap.offset + wi * BPR * D,
                                  [m_ap.ap[0], [GRP * BPR * D, NRANGE],
                                   [D, BPR], [1, D]])
                        ew_ap = ew_t[:, wg * BPW:(wg + 1) * BPW]
                        in_ew = AP(ew_ap.tensor, ew_ap.offset,
                                   [ew_ap.ap[0], [BPR, NRANGE], [1, BPR], [0, D]])
                        sc_ap = sc[:]
                        out_sc = AP(sc_ap.tensor, sc_ap.offset,
                                    [sc_ap.ap[0], [BPR * D, NRANGE],
                                     [D, BPR], [1, D]])
                        nc.vector.tensor_tensor(out=out_sc, in0=in_m, in1=in_ew,
                                                op=Alu.mult)
                        psum_w = ppool.tile([D, W], f32, tag="agg")
                        for j in range(BPW):
                            nc.tensor.matmul(
                                out=psum_w[:],
                                lhsT=sc[:, j * D:(j + 1) * D],
                                rhs=oh[:, j * W:(j + 1) * W],
                                start=(j == 0), stop=(j == BPW - 1))
                        if wg % 2 == 0:
                            aggT = wpool.tile([D, 128], f32, tag="aggT")
                        nc.vector.tensor_copy(
                            out=aggT[:, (wg % 2) * W:(wg % 2 + 1) * W],
                            in_=psum_w[:])
                        if wg % 2 == 0:
                            continue
                        # ---- transform + elu for tile t ----
                        t = wg // 2
                        xT = wpool.tile([D, 128], f32, tag="xT")
                        nc.sync.dma_start(out=xT[:], in_=xT_ap(L, t))
                        pt = ppool.tile([od, 128], f32, tag="tr")
                        nc.tensor.matmul(out=pt[:], lhsT=wrel_t[L][:],
                                         rhs=aggT[:], start=True, stop=False)
                        nc.tensor.matmul(out=pt[:], lhsT=wroot_t[L][:],
                                         rhs=xT[:], start=False, stop=True)
                        mm = wpool.tile([od, 128], f32, tag="mm")
                        nc.vector.tensor_scalar(out=mm[:], in0=pt[:],
                                             scalar1=bias_t[L][:], scalar2=0.0,
                                             op0=Alu.add, op1=Alu.min)
                        ee = wpool.tile([od, 128], f32, tag="ee")
                        nc.scalar.activation(out=ee[:], in_=mm[:], func=Act.Exp)
                        rr = wpool.tile([od, 128], f32, tag="rr")
                        nc.vector.tensor_scalar(out=rr[:], in0=pt[:],
                                             scalar1=bias_t[L][:], scalar2=0.0,
                                             op0=Alu.add, op1=Alu.max)
                        hT = wpool.tile([od, 128], f32, tag="hT")
                        nc.vector.tensor_tensor(out=hT[:], in0=ee[:], in1=rr[:],
                                                op=Alu.add)
                        nc.vector.tensor_scalar(out=hT[:], in0=hT[:], scalar1=-1.0,
                                             scalar2=None, op0=Alu.add)
                        if L < 4:
                            nc.sync.dma_start(
                                out=hT_own[L][:, t * 128:(t + 1) * 128],
                                in_=hT[:])
                            tp = ppool.tile([128, D], f32, tag="tp")
                            nc.tensor.transpose(out=tp[:], in_=hT[:],
                                                identity=ident_t[:])
                            hn = wpool.tile([128, D], f32, tag="hn")
                            nc.vector.tensor_copy(out=hn[:], in_=tp[:])
                            nc.sync.dma_start(
                                out=stages[L][t * 128:(t + 1) * 128, :],
                                in_=hn[:])
                        else:
                            nc.sync.dma_start(
                                out=h5_stage[:, t * 128:(t + 1) * 128],
                                in_=hT[:])
                        # fire AllGather halves as soon as available
                        if L < 4 and t == WPC // 4 - 1:
                            _ag(nc, collectives, stages[L][:, :], tables[L + 1],
                                0, dpool, L)
                        if L < 4 and t == WPC // 2 - 1:
                            _ag(nc, collectives, stages[L][:, :], tables[L + 1],
                                1, dpool, L)
                # end groups
            for L in range(4):
                nc.gpsimd.dma_start(out=h_out[L][:, :], in_=stages[L][:])
            nc.gpsimd.dma_start(out=h_out[4][:, :], in_=h5_stage[:])

    nc.compile()
    return nc


def _ag(nc, collectives, stage, table_halves, half, dpool, L):
    """AllGather stage rows [half*6272,(half+1)*6272) -> full half-table."""
    import concourse.mybir as mybir
    rows = HALF
    src = stage[half * rows:(half + 1) * rows, :]
    dst_tile = table_halves[half]
    if collectives:
        nc.gpsimd.collective_compute(
            "AllGather", mybir.AluOpType.bypass,
            replica_groups=[list(range(NC))],
            ins=[src.opt()], outs=[dst_tile.opt()])
    else:
        # sim mode: local copy of own shard (timing proxy, wrong data)
        nc.gpsimd.dma_start(out=dst_tile[:rows, :], in_=src)


# ---------------- head (numpy) ----------------
def _elu(x):
    return np.where(x > 0, x, np.expm1(x))


def _head(xcat, batch, num_graphs, conv1_w, conv1_b, conv2_w, conv2_b,
          mlp_w1, mlp_b1, mlp_w2, mlp_b2):
    n, d = xcat.shape
    perm = np.lexsort((-xcat[:, -1], batch))
    xs = xcat[perm]
    bs = batch[perm]
    counts = np.zeros(num_graphs, np.int64)
    np.add.at(counts, batch, 1)
    starts = np.cumsum(counts) - counts
    rank = np.arange(n) - starts[bs]
    slot = np.minimum(rank, K)
    out = np.zeros((num_graphs, K + 1, d), xcat.dtype)
    out[bs, slot] = xs
    xp = out[:, :K]
    h = _elu(np.einsum('bkd,od->bok', xp, conv1_w) + conv1_b[None, :, None])
    b_, c_, l_ = h.shape
    h = h.reshape(b_, c_, l_ // 2, 2).max(axis=-1)
    B, C, L = h.shape
    O, I, KK = conv2_w.shape
    outc = np.zeros((B, O, L - KK + 1), np.float32)
    for k in range(KK):
        outc += np.einsum('bcl,oc->bol', h[:, :, k:k + L - KK + 1],
                          conv2_w[:, :, k])
    h = _elu(outc + conv2_b[None, :, None])
    h = h.reshape(num_graphs, -1)
    h = _elu(h @ mlp_w1.T + mlp_b1)
    return (h @ mlp_w2.T + mlp_b2).astype(np.float32)


# ---------------- entry point ----------------
def kernel(x, edge_index, batch, edge_weight, num_graphs, gc_w_rel, gc_b_rel,
           gc_w_root, conv1_w, conv1_b, conv2_w, conv2_b, mlp_w1, mlp_b1,
           mlp_w2, mlp_b2):
    from concourse import bass_utils

    x = np.asarray(x, dtype=np.float32)
    batch_np = np.asarray(batch, dtype=np.int64)
    ng = int(num_graphs)

    idx_w, relw, eww = _prep_edges(np.asarray(edge_index), np.asarray(edge_weight))
    xs, xT = _node_tables(x)
    iota = np.tile(np.arange(W, dtype=np.float32)[None, :], (128, 1))

    if "nc" not in _cache:
        _cache["nc"] = _build_program(collectives=True)
    nc = _cache["nc"]

    in_maps = []
    for cc in range(NC):
        m = dict(x_stage=xs[cc], xT_own=xT[cc], idx16=idx_w[cc],
                 reldst=relw[cc], eww=eww[cc], iota=iota,
                 ident64=np.eye(D, dtype=np.float32))
        for L in range(NLAYERS):
            m[f"wrelT{L}"] = np.ascontiguousarray(
                np.asarray(gc_w_rel[L], np.float32).T)
            m[f"wrootT{L}"] = np.ascontiguousarray(
                np.asarray(gc_w_root[L], np.float32).T)
            m[f"bias{L}"] = np.asarray(gc_b_rel[L], np.float32)[:, None]
        in_maps.append(m)

    res = bass_utils.run_bass_kernel_spmd(nc, in_maps,
                                          core_ids=list(range(NC)))

    # assemble xcat [100000, 257]
    xcat = np.zeros((N_NODES, 4 * D + 1), dtype=np.float32)
    for cc in range(NC):
        r = res.results[cc]
        sl = slice(cc * NPC_REAL, (cc + 1) * NPC_REAL)
        for L in range(4):
            xcat[sl, L * D:(L + 1) * D] = r[f"h{L}_out"][:NPC_REAL]
        xcat[sl, 4 * D] = r["h4_out"][0, :NPC_REAL]

    out = _head(xcat, batch_np, ng,
                np.asarray(conv1_w, np.float32), np.asarray(conv1_b, np.float32),
                np.asarray(conv2_w, np.float32), np.asarray(conv2_b, np.float32),
                np.asarray(mlp_w1, np.float32), np.asarray(mlp_b1, np.float32),
                np.asarray(mlp_w2, np.float32), np.asarray(mlp_b2, np.float32))
    return out
